# revision 1
# baseline (speedup 1.0000x reference)
"""GAT message-passing kernel for 8 Trainium2 NeuronCores.

Strategy (edge-parallel by dst-range, no collectives):
  - Host: sort edges by dst; core c owns dst nodes [c*12500, (c+1)*12500).
    Within a core, dst nodes are tiled 128 at a time; each tile's edges are
    split into chunks of 128 (padded; chunk count per tile = max over cores
    so the SPMD instruction stream is identical on all cores).
  - Device, per chunk of 128 edges (edges on partitions):
      hk_g   [128e, 64]  <- indirect DMA gather of hk[src]
      hk_gT  [64, 128e]  <- PE transpose
      S.T    [128e,128d] <- matmul(lhsT=hk_gT, rhs=huT_tile)   (scores, fp32)
      expS   [128e,128d] <- ACT exp -> bf16 (no max-subtraction needed:
                            |score| <~ 45 so exp stays finite in fp32)
      P.T    [128e,128d] <- expS * onehot(local_dst == iota)   (bf16)
      rst    [128d, 65]  += P.T^T @ [hk_g_bf16 | 1]            (PSUM accum)
    Per dst-tile epilogue: alpha-normalize by column 64 (the segment sum),
    PE transpose, FC matmul with host-prepared [W^T; b] (bias via ones row),
    ReLU, DMA out.
"""
import contextlib
import sys

for p in ("/opt/trn_rl_repo",):
    if p not in sys.path:
        sys.path.insert(0, p)

import numpy as np
import concourse.bass as bass
import concourse.tile as tile
from concourse import mybir, bacc
from concourse.bass_utils import run_bass_kernel_spmd
from concourse.masks import make_identity

f32 = mybir.dt.float32
bf16 = mybir.dt.bfloat16
i32 = mybir.dt.int32

N_CORES = 8
P = 128


def _tile_body(nc, t, gt, goff, n_nodes_core, d_feat,
               hk, y, hut_sb, sidx_sb, ldst_sb, wt_sb, iota_sb, ident,
               pool, epool, ps_st, ps_tr, ps_rst, ps_epi, ablate,
               shared_hkg=None, sink=None):
    hut_t = hut_sb[:, t * P:(t + 1) * P]
    rst_ps = ps_rst.tile([P, d_feat + 1], f32, tag="rst")
    for g in range(gt):
        col = goff + g
        if ablate == "compute_only":
            hk_g = shared_hkg
        else:
            hk_g = pool.tile([P, d_feat], f32, tag="hk_g")
            nc.gpsimd.indirect_dma_start(
                out=hk_g[:], out_offset=None, in_=hk.ap(),
                in_offset=bass.IndirectOffsetOnAxis(
                    ap=sidx_sb[:, col:col + 1], axis=0))
        if ablate == "gather_only":
            # keep gathers live: fold each into a persistent sink
            r = pool.tile([P, 1], f32, tag="gsink")
            nc.vector.tensor_reduce(out=r[:], in_=hk_g[:],
                                    axis=mybir.AxisListType.X,
                                    op=mybir.AluOpType.max)
            nc.vector.tensor_tensor(out=sink[:], in0=sink[:], in1=r[:],
                                    op=mybir.AluOpType.max)
            continue
        hkT_ps = ps_tr.tile([d_feat, P], f32, tag="hkT")
        nc.tensor.transpose(out=hkT_ps[:], in_=hk_g[:], identity=ident[:])
        hkT = pool.tile([d_feat, P], f32, tag="hkT_sb")
        nc.vector.tensor_copy(out=hkT[:], in_=hkT_ps[:])

        st_ps = ps_st.tile([P, P], f32, tag="st")
        nc.tensor.matmul(out=st_ps[:], lhsT=hkT[:], rhs=hut_t,
                         start=True, stop=True)
        exps = pool.tile([P, P], bf16, tag="exps")
        nc.scalar.activation(exps[:], st_ps[:],
                             mybir.ActivationFunctionType.Exp)
        onehot = pool.tile([P, P], bf16, tag="onehot")
        nc.vector.tensor_tensor(
            out=onehot[:],
            in0=ldst_sb[:, col:col + 1].to_broadcast([P, P]),
            in1=iota_sb[:],
            op=mybir.AluOpType.is_equal)
        pt = pool.tile([P, P], bf16, tag="pt")
        nc.vector.tensor_tensor(out=pt[:], in0=exps[:], in1=onehot[:],
                                op=mybir.AluOpType.mult)
        vals = pool.tile([P, d_feat + 1], bf16, tag="vals")
        nc.vector.tensor_copy(out=vals[:, 0:d_feat], in_=hk_g[:])
        nc.vector.memset(vals[:, d_feat:d_feat + 1], 1.0)
        nc.tensor.matmul(out=rst_ps[:], lhsT=pt[:], rhs=vals[:],
                         start=(g == 0), stop=(g == gt - 1))

    if ablate == "gather_only":
        return
    # epilogue: normalize, transpose, FC, relu, store
    denom = epool.tile([P, 1], f32, tag="denom")
    nc.vector.tensor_scalar_add(denom[:], rst_ps[:, d_feat:d_feat + 1], 1e-30)
    recip = epool.tile([P, 1], f32, tag="recip")
    nc.vector.reciprocal(recip[:], denom[:])
    rst_sb = epool.tile([P, d_feat + 1], f32, tag="rst_sb")
    nc.vector.tensor_scalar_mul(rst_sb[:, 0:d_feat], rst_ps[:, 0:d_feat],
                                recip[:])
    nc.vector.memset(rst_sb[:, d_feat:d_feat + 1], 1.0)

    rstT_ps = ps_epi.tile([d_feat + 1, P], f32, tag="rstT")
    nc.tensor.transpose(out=rstT_ps[:], in_=rst_sb[:], identity=ident[:])
    rstT = epool.tile([d_feat + 1, P], f32, tag="rstT_sb")
    nc.vector.tensor_copy(out=rstT[:], in_=rstT_ps[:])

    out_ps = ps_epi.tile([P, wt_sb.shape[1]], f32, tag="out_ps")
    nc.tensor.matmul(out=out_ps[:], lhsT=rstT[:], rhs=wt_sb[:],
                     start=True, stop=True)
    out_sb = epool.tile([P, wt_sb.shape[1]], f32, tag="out_sb")
    nc.scalar.activation(out_sb[:], out_ps[:],
                         mybir.ActivationFunctionType.Relu)
    rows = min(P, n_nodes_core - t * P)
    nc.sync.dma_start(y.ap()[t * P:t * P + rows], out_sb[:rows])


def build_gat_kernel(n_nodes_core, n_tiles, g_list, nk_rows, d_feat, d_out,
                     repeat=1, ablate=None):
    """Build the per-core SPMD kernel. g_list[t] = #128-edge chunks in tile t."""
    sum_g = sum(g_list)
    pad_nodes = n_tiles * P
    nc = bacc.Bacc("TRN2", target_bir_lowering=False, debug=False,
                   num_devices=N_CORES)
    hk = nc.dram_tensor("hk", [nk_rows, d_feat], f32, kind="ExternalInput")
    hut = nc.dram_tensor("hut", [d_feat, pad_nodes], f32, kind="ExternalInput")
    srcidx = nc.dram_tensor("srcidx", [P, sum_g], i32, kind="ExternalInput")
    ldst = nc.dram_tensor("ldst", [P, sum_g], f32, kind="ExternalInput")
    wt_aug = nc.dram_tensor("wt_aug", [d_feat + 1, d_out], f32,
                            kind="ExternalInput")
    iota_row = nc.dram_tensor("iota_row", [P, P], f32, kind="ExternalInput")
    y = nc.dram_tensor("y", [n_nodes_core, d_out], f32, kind="ExternalOutput")

    with tile.TileContext(nc) as tc:
        with (
            tc.tile_pool(name="const", bufs=1) as cpool,
            tc.tile_pool(name="work", bufs=4) as pool,
            tc.tile_pool(name="epi", bufs=2) as epool,
            tc.tile_pool(name="ps_st", bufs=2, space="PSUM") as ps_st,
            tc.tile_pool(name="ps_tr", bufs=2, space="PSUM") as ps_tr,
            tc.tile_pool(name="ps_rst", bufs=2, space="PSUM") as ps_rst,
            tc.tile_pool(name="ps_epi", bufs=1, space="PSUM") as ps_epi,
        ):
            ident = cpool.tile([P, P], f32)
            make_identity(nc, ident[:])
            wt_sb = cpool.tile([d_feat + 1, d_out], f32)
            nc.sync.dma_start(wt_sb[:], wt_aug.ap())
            iota_sb = cpool.tile([P, P], f32)
            nc.sync.dma_start(iota_sb[:], iota_row.ap())
            hut_sb = cpool.tile([d_feat, pad_nodes], f32)
            nc.sync.dma_start(hut_sb[:], hut.ap())
            sidx_sb = cpool.tile([P, sum_g], i32)
            nc.sync.dma_start(sidx_sb[:], srcidx.ap())
            ldst_sb = cpool.tile([P, sum_g], f32)
            nc.sync.dma_start(ldst_sb[:], ldst.ap())

            shared_hkg = None
            sink = None
            if ablate == "compute_only":
                shared_hkg = cpool.tile([P, d_feat], f32, tag="shared_hkg")
                nc.vector.memset(shared_hkg[:], 0.01)
            if ablate == "gather_only":
                sink = cpool.tile([P, 1], f32, tag="sink")
                nc.vector.memset(sink[:], 0.0)

            loop_cm = (tc.For_i(0, repeat, 1) if repeat > 1
                       else contextlib.nullcontext())
            with loop_cm:
                goff = 0
                for t in range(n_tiles):
                    _tile_body(nc, t, g_list[t], goff, n_nodes_core, d_feat,
                               hk, y, hut_sb, sidx_sb, ldst_sb, wt_sb,
                               iota_sb, ident, pool, epool, ps_st, ps_tr,
                               ps_rst, ps_epi, ablate, shared_hkg, sink)
                    goff += g_list[t]
            if ablate == "gather_only":
                nc.sync.dma_start(y.ap()[0:1, 0:1], sink[0:1, 0:1])
    nc.compile()
    return nc


def prep_inputs(hk, hu, W, b, src, dst, n_cores=N_CORES):
    """Host-side sharding prep. Returns (per-core in_maps, g_list, meta)."""
    n_nodes, d_feat = hk.shape
    d_out = W.shape[0]
    npc = n_nodes // n_cores          # nodes per core
    n_tiles = (npc + P - 1) // P
    pad_nodes = n_tiles * P

    src = np.ascontiguousarray(src.astype(np.int32))
    dst = np.ascontiguousarray(dst.astype(np.int32))
    order = np.argsort(dst, kind="stable")
    dst_s = dst[order]
    src_s = src[order]

    # edge count per (core, tile): tiles are 128-node blocks LOCAL to each
    # core's [c*npc, (c+1)*npc) range (npc need not be a multiple of 128).
    core_of = dst_s // npc
    local_tile = (dst_s - core_of * npc) // P
    flat = core_of * n_tiles + local_tile
    counts = np.bincount(flat, minlength=n_cores * n_tiles)
    counts = counts.reshape(n_cores, n_tiles)
    g_list = np.maximum(1, (counts.max(axis=0) + P - 1) // P).astype(int).tolist()
    sum_g = int(sum(g_list))

    starts = np.zeros(n_cores * n_tiles + 1, np.int64)
    np.cumsum(counts.reshape(-1), out=starts[1:])

    wt_aug = np.concatenate([W.T, b[None, :]], axis=0).astype(np.float32)
    iota_row = np.tile(np.arange(P, dtype=np.float32), (P, 1))
    hk = np.ascontiguousarray(hk, np.float32)

    in_maps = []
    goffs = np.concatenate([[0], np.cumsum(g_list)]).astype(int)
    for c in range(n_cores):
        srcidx = np.zeros((P, sum_g), np.int32)
        ldst_arr = np.full((P, sum_g), 999.0, np.float32)
        for t in range(n_tiles):
            gtile = c * n_tiles + t
            s, e = starts[gtile], starts[gtile + 1]
            cnt = e - s
            if cnt == 0:
                continue
            go = goffs[t]
            j = np.arange(cnt)
            pp = j % P
            gg = j // P
            srcidx[pp, go + gg] = src_s[s:e]
            ldst_arr[pp, go + gg] = (dst_s[s:e] - (c * npc + t * P)).astype(
                np.float32)
        hut = np.zeros((d_feat, pad_nodes), np.float32)
        hut[:, :npc] = hu[c * npc:(c + 1) * npc].T
        in_maps.append({
            "hk": hk, "hut": hut, "srcidx": srcidx, "ldst": ldst_arr,
            "wt_aug": wt_aug, "iota_row": iota_row,
        })
    meta = dict(npc=npc, n_tiles=n_tiles, n_nodes=n_nodes, d_feat=d_feat,
                d_out=d_out)
    return in_maps, g_list, meta


_KERNEL_CACHE = {}


def run_gat(hk, hu, W, b, src, dst, n_cores=N_CORES, repeat=1, ablate=None):
    in_maps, g_list, meta = prep_inputs(hk, hu, W, b, src, dst, n_cores)
    key = (tuple(g_list), meta["npc"], meta["d_feat"], meta["d_out"],
           hk.shape[0], repeat, ablate)
    if key not in _KERNEL_CACHE:
        _KERNEL_CACHE[key] = build_gat_kernel(
            meta["npc"], meta["n_tiles"], g_list, hk.shape[0],
            meta["d_feat"], meta["d_out"], repeat=repeat, ablate=ablate)
    nc = _KERNEL_CACHE[key]
    res = run_bass_kernel_spmd(nc, in_maps, core_ids=list(range(n_cores)))
    out = np.concatenate([res.results[c]["y"] for c in range(n_cores)], axis=0)
    return np.ascontiguousarray(out, np.float32)


def kernel(hk, hu, W, b, src, dst):
    hk = np.asarray(hk, np.float32)
    hu = np.asarray(hu, np.float32)
    W = np.asarray(W, np.float32)
    b = np.asarray(b, np.float32)
    return run_gat(hk, hu, W, b, np.asarray(src), np.asarray(dst))



# revision 3
# speedup vs baseline: 6139.4310x; 6139.4310x over previous
"""GAT message-passing kernel v2 for 8 Trainium2 NeuronCores.

Design (edge-parallel by dst-range, no collectives):
  Host: sort edges by dst; core c owns dst nodes [c*12500, (c+1)*12500).
  Within a core, dst tiles of 128 nodes; edges of a tile are split by src
  bank (4 banks of 25600 rows, int16-indexable) into static chunk counts
  g[t][b] = max over cores of ceil(count/128).

  Gather: dma_gather (SWDGE Q7 batch gather), one call per (supertile of 2
  dst tiles, bank), ~1024 idxs/call. Pads cycle rows 1..128 (constant-value
  pad runs wedge the SDMA; padded rows are masked out downstream). Output
  [128, chunks, 64] f32: edge j of a call lands at (partition j%128,
  col j//128).

  Compute per chunk of 128 edges (dst tile T):
    PE transpose gather slice [128e,64] -> hkT psum (f32), pairs share one
      [128,256] DVE copy -> fp16 megapair (chunk A partitions 0:64, B 64:128)
    mm1a: st[e,d] = hkT^T @ hut_tile (fp16, f32 psum)
    mm1b: += maskT_chunk^T @ maskd  (bit-match mask fold: rows 0:7 C*bit_b(
      ldst), 7:14 C*(1-bit_b); matching dst -> +7C, else <= +6C)
    exp (per pair): pt = exp(st - 7C) bf16  -> masked softmax numerators
    vals: copy gather slice f32->bf16 into ring tile with ones col 64
    mm2: rst[128d, 65] += pt^T @ vals  (col 64 = denominator)
  Epilogue per tile: den+=eps, recip, transpose rst, FC matmul with
  [W^T; b] (bias row scales by den so normalize folds after FC), ACT
  relu(out * recip), DMA out.
"""
import contextlib
import sys

for p in ("/opt/trn_rl_repo",):
    if p not in sys.path:
        sys.path.insert(0, p)

import numpy as np
import concourse.bass as bass
import concourse.tile as tile
from concourse import mybir, bacc
from concourse.bass_utils import run_bass_kernel_spmd
from concourse.masks import make_identity

f32 = mybir.dt.float32
f16 = mybir.dt.float16
bf16 = mybir.dt.bfloat16
i16 = mybir.dt.int16

N_CORES = 8
P = 128
N_NODES = 100000
D_FEAT = 64
D_OUT = 128
NPC = N_NODES // N_CORES          # 12500
N_TILES = (NPC + P - 1) // P      # 98
PAD_NODES = N_TILES * P           # 12544
BANKS = 4
BANK_SZ = 25600
ST = 2                            # dst tiles per gather supertile
N_ST = (N_TILES + ST - 1) // ST   # 49
C_MASK = 100.0


def build_v2(g, repeat=1, ablate=None):
    """g: [N_TILES][BANKS] static chunk counts (python ints)."""
    g = [[int(x) for x in row] for row in g]
    tile_chunks = [sum(row) for row in g]
    totc = sum(tile_chunks)
    max_tc = max(tile_chunks)
    # per-bank max chunks within any supertile call
    call_chunks = [[g[2 * s][b] + (g[2 * s + 1][b] if 2 * s + 1 < N_TILES
                                   else 0) for b in range(BANKS)]
                   for s in range(N_ST)]
    maxcall = [max(call_chunks[s][b] for s in range(N_ST))
               for b in range(BANKS)]
    tot_idx = sum(call_chunks[s][b] * P for s in range(N_ST)
                  for b in range(BANKS))

    nc = bacc.Bacc("TRN2", target_bir_lowering=False, debug=False,
                   num_devices=N_CORES)
    hkov = nc.dram_tensor("hkov", [N_NODES, 2 * D_FEAT], f16,
                          kind="ExternalInput")
    idxs = nc.dram_tensor("idxs", [P, tot_idx // 16], i16,
                          kind="ExternalInput")
    maskt = nc.dram_tensor("maskt", [14, tot_idx], f16, kind="ExternalInput")
    hut = nc.dram_tensor("hut", [D_FEAT + 14, PAD_NODES], f16,
                         kind="ExternalInput")
    wtaug = nc.dram_tensor("wtaug", [D_FEAT + 1, D_OUT], bf16,
                           kind="ExternalInput")
    y = nc.dram_tensor("y", [NPC, D_OUT], f32, kind="ExternalOutput")

    with tile.TileContext(nc) as tc:
        with (
            tc.tile_pool(name="const", bufs=1) as cpool,
            tc.tile_pool(name="epi", bufs=2) as epool,
            tc.tile_pool(name="ps_tr", bufs=2, space="PSUM") as ps_tr,
            tc.tile_pool(name="ps_st", bufs=2, space="PSUM") as ps_st,
            tc.tile_pool(name="ps_rst", bufs=2, space="PSUM") as ps_rst,
            tc.tile_pool(name="ps_epi", bufs=1, space="PSUM") as ps_epi,
        ):
            ident = cpool.tile([P, P], f32)
            make_identity(nc, ident[:])
            identh = cpool.tile([P, P], f16)
            make_identity(nc, identh[:])
            hut_sb = cpool.tile([D_FEAT + 14, PAD_NODES], f16)
            nc.sync.dma_start(hut_sb[:], hut.ap())
            wt_sb = cpool.tile([D_FEAT + 1, D_OUT], bf16)
            nc.sync.dma_start(wt_sb[:], wtaug.ap())
            idx_sb = cpool.tile([P, tot_idx // 16], i16)
            nc.sync.dma_start(idx_sb[:], idxs.ap())
            bias_t = cpool.tile([P, 1], f32)
            nc.vector.memset(bias_t[:], -7.0 * C_MASK)

            # manual rings (memset once -> no stale-NaN on skipped slots)
            NG = 3
            G = [[cpool.tile([P, maxcall[b] * P], f16,
                             tag=f"G{r}b{b}", name=f"G{r}b{b}")
                  for b in range(BANKS)] for r in range(NG)]
            for r in range(NG):
                for b in range(BANKS):
                    nc.vector.memset(G[r][b][:], 0.0)

            NVAL = 6
            V = [cpool.tile([P, D_FEAT + 1], bf16, tag=f"V{i}",
                            name=f"V{i}") for i in range(NVAL)]
            for i in range(NVAL):
                nc.vector.memset(V[i][:], 1.0)

            NPT = 3
            PT = [cpool.tile([P, 2 * P], bf16, tag=f"PT{i}",
                             name=f"PT{i}") for i in range(NPT)]
            GFAKE = None
            if ablate == "decoupled":
                GFAKE = cpool.tile([P, 16 * D_FEAT], f32, name="GFAKE")
                nc.vector.memset(GFAKE[:], 0.01)

            idx_off = 0        # in idx columns (16 idxs per column)
            call_off = [[0] * BANKS for _ in range(N_ST)]
            o = 0
            for s in range(N_ST):
                for b in range(BANKS):
                    call_off[s][b] = o
                    o += call_chunks[s][b] * P // 16

            vi = [0]           # vals ring cursor
            mi = [0]           # mega ring cursor
            pi = [0]           # pt ring cursor

            def do_tile(T, s):
                """Process dst tile T inside supertile s."""
                rst = ps_rst.tile([P, D_FEAT + 1], f32, tag="rst")
                # chunk list (bank-major); pairs share one exp
                chunks = []
                for b in range(BANKS):
                    base = g[2 * s][b] if T % 2 == 1 else 0
                    for c in range(g[T][b]):
                        chunks.append((b, base + c))
                n_ch = len(chunks)
                for i0 in range(0, n_ch, 2):
                    pair = chunks[i0:i0 + 2]
                    npair = len(pair)
                    st_ps = ps_st.tile([P, 2 * P], f32, tag="st")
                    for k, (b2, col) in enumerate(pair):
                        gt = G[s % NG][b2]
                        nc.tensor.matmul(
                            out=st_ps[:, k * P:(k + 1) * P],
                            lhsT=gt[0:78, col * P:(col + 1) * P],
                            rhs=hut_sb[:, T * P:(T + 1) * P],
                            start=True, stop=True)
                    pt = PT[pi[0] % NPT]; pi[0] += 1
                    nc.scalar.activation(pt[:, 0:npair * P],
                                         st_ps[:, 0:npair * P],
                                         mybir.ActivationFunctionType.Exp,
                                         bias=bias_t[:])
                    for k, (b2, col) in enumerate(pair):
                        cc = i0 + k
                        gt = G[s % NG][b2]
                        tv = ps_tr.tile([P, P], f16, tag="tr")
                        nc.tensor.transpose(
                            out=tv[:], in_=gt[:, col * P:(col + 1) * P],
                            identity=identh[:])
                        v = V[vi[0] % NVAL]; vi[0] += 1
                        if cc % 2 == 0:
                            nc.vector.tensor_copy(out=v[:, 0:D_FEAT],
                                                  in_=tv[:, 0:D_FEAT])
                        else:
                            nc.scalar.copy(out=v[:, 0:D_FEAT],
                                           in_=tv[:, 0:D_FEAT])
                        nc.tensor.matmul(out=rst[:],
                                         lhsT=pt[:, k * P:(k + 1) * P],
                                         rhs=v[:], start=(cc == 0),
                                         stop=(cc == n_ch - 1))
                # epilogue
                rst_sb = epool.tile([P, D_FEAT + 1], f32, tag="rst_sb")
                nc.vector.tensor_copy(out=rst_sb[:], in_=rst[:])
                nc.vector.tensor_scalar_add(rst_sb[:, D_FEAT:D_FEAT + 1],
                                            rst_sb[:, D_FEAT:D_FEAT + 1],
                                            1e-30)
                recip = epool.tile([P, 1], f32, tag="recip")
                nc.vector.reciprocal(recip[:], rst_sb[:, D_FEAT:D_FEAT + 1])
                rstT_ps = ps_epi.tile([D_FEAT + 1, P], f32, tag="rstT")
                nc.tensor.transpose(out=rstT_ps[:], in_=rst_sb[:],
                                    identity=ident[:])
                rstT = epool.tile([D_FEAT + 1, P], bf16, tag="rstT_sb")
                nc.vector.tensor_copy(out=rstT[:], in_=rstT_ps[:])
                out_ps = ps_epi.tile([P, D_OUT], f32, tag="out_ps")
                nc.tensor.matmul(out=out_ps[:], lhsT=rstT[:], rhs=wt_sb[:],
                                 start=True, stop=True)
                out_sb = epool.tile([P, D_OUT], f32, tag="out_sb")
                nc.scalar.activation(out_sb[:], out_ps[:],
                                     mybir.ActivationFunctionType.Relu,
                                     scale=recip[:])
                rows = min(P, NPC - T * P)
                nc.sync.dma_start(y.ap()[T * P:T * P + rows], out_sb[:rows])

            if True:
                pass
            loop_cm = (tc.For_i(0, repeat, 1) if repeat > 1
                       else contextlib.nullcontext())
            def emit_gathers(s):
                if ablate in ("no_gather", "decoupled_ng"):
                    return
                for b in range(BANKS):
                    nch = call_chunks[s][b]
                    if nch == 0:
                        continue
                    gt = G[s % NG][b]
                    nc.gpsimd.dma_gather(
                        out_ap=gt[:, 0:nch * P].rearrange(
                            "p (o c) -> p o c", o=1),
                        in_ap=hkov.ap()[b * BANK_SZ:
                                        min((b + 1) * BANK_SZ, N_NODES)],
                        idxs_ap=idx_sb[:, call_off[s][b]:
                                       call_off[s][b] + nch * P // 16],
                        num_idxs=nch * P,
                        num_idxs_reg=nch * P,
                        elem_size=2 * D_FEAT,
                        transpose=True,
                        single_packet=False,
                    )
                    nc.sync.dma_start(
                        gt[64:78, 0:nch * P],
                        maskt.ap()[:, call_off[s][b] * 16:
                                   call_off[s][b] * 16 + nch * P])

            with loop_cm:
                # software pipeline: emit gathers one supertile ahead so
                # coarse (tick-based) WAR waits land on compute(s-1), letting
                # gathers(s+1) run during compute(s).
                emit_gathers(0)
                emit_gathers(1)
                for s in range(N_ST):
                    if s + 2 < N_ST:
                        emit_gathers(s + 2)
                    for t in range(ST):
                        T = 2 * s + t
                        if T < N_TILES and ablate != "gather_only":
                            do_tile(T, s)
                if ablate == "gather_only":
                    snk = epool.tile([P, D_FEAT], f32, tag="snk")
                    nc.vector.tensor_copy(out=snk[:], in_=G[0][0][:, 0:D_FEAT])
                    nc.sync.dma_start(y.ap()[0:P, 0:D_FEAT], snk[:])
    nc.compile()
    return nc


def prep_inputs_v2(hk, hu, W, b, src, dst):
    """Host-side sharding prep. Returns (per-core in_maps, g, meta)."""
    src = np.asarray(src).astype(np.int64)
    dst = np.asarray(dst).astype(np.int64)
    hk = np.ascontiguousarray(hk, np.float32)
    hu = np.asarray(hu, np.float32)

    order = np.argsort(dst, kind="stable")
    src_s = src[order]
    dst_s = dst[order]
    core = dst_s // NPC
    local = dst_s - core * NPC
    tl = local // P
    ldst = (local % P).astype(np.int64)
    bank = src_s // BANK_SZ
    lidx = (src_s - bank * BANK_SZ).astype(np.int64)

    key = (core * N_TILES + tl) * BANKS + bank
    ord2 = np.argsort(key, kind="stable")
    key_s = key[ord2]
    lidx_s = lidx[ord2]
    ldst_s = ldst[ord2]

    counts = np.bincount(key_s, minlength=N_CORES * N_TILES * BANKS)
    counts = counts.reshape(N_CORES, N_TILES, BANKS)
    g = np.maximum(1, -(-counts.max(axis=0) // P))       # [N_TILES, BANKS]
    tile_chunks = g.sum(axis=1)                          # [N_TILES]
    totc = int(tile_chunks.sum())

    # supertile call sizes and in-call tile segment bases
    call_chunks = np.zeros((N_ST, BANKS), np.int64)
    for s in range(N_ST):
        call_chunks[s] = g[2 * s] + (g[2 * s + 1] if 2 * s + 1 < N_TILES
                                     else 0)
    tot_idx = int(call_chunks.sum() * P)

    # per-edge slot: position j within its (core,tile,bank) run
    starts = np.zeros(N_CORES * N_TILES * BANKS + 1, np.int64)
    np.cumsum(counts.reshape(-1), out=starts[1:])
    j_in_run = np.arange(len(key_s)) - starts[key_s]

    t_of = (key_s // BANKS) % N_TILES
    b_of = key_s % BANKS
    c_of = key_s // (N_TILES * BANKS)

    # in-call idx position: tile segment base + j
    s_of = t_of // ST
    seg_base = np.where(t_of % ST == 1, g[(t_of // ST) * ST, b_of] * P, 0)
    call_pos = seg_base + j_in_run
    # call base offset in the global idx stream (st-major, bank minor)
    call_base = np.concatenate(
        [[0], np.cumsum(call_chunks.reshape(-1) * P)])[:-1].reshape(
        N_ST, BANKS)
    idx_pos = call_base[s_of, b_of] + call_pos
    mask_pos = idx_pos          # mask stream is call-ordered now

    # ---- build per-core arrays
    in_maps = []
    maskd = np.zeros((14, P), np.float16)
    d_ar = np.arange(P)
    for bb in range(7):
        maskd[bb] = ((d_ar >> bb) & 1).astype(np.float16)
        maskd[7 + bb] = 1.0 - maskd[bb]
    wtaug = np.concatenate([W.T, b[None, :]], axis=0)
    import ml_dtypes
    wtaug = wtaug.astype(ml_dtypes.bfloat16)
    hkf = hk.astype(np.float16)
    hkov = np.zeros((N_NODES, 2 * D_FEAT), np.float16)
    hkov[:, :D_FEAT] = hkf
    hkov[:-1, D_FEAT:] = hkf[1:]

    # idx stream template: ALL pads are varied safe indices. Long runs of a
    # constant pad value (0 or -1) wedge the SDMA engines (HW-observed), so
    # pads cycle through rows 1..128; they gather junk that the zero mask
    # rows suppress.
    idx_template = ((np.arange(tot_idx) % P) + 1).astype(np.int16)

    for c in range(N_CORES):
        m = c_of == c
        idx_arr = idx_template.copy()
        idx_arr[idx_pos[m]] = lidx_s[m].astype(np.int16)
        # wrap [16, n/16] per call then concat: since calls are contiguous
        # 128-multiples, a global wrap of each call block:
        wrapped = np.empty((16, tot_idx // 16), np.int16)
        o16 = 0
        for s in range(N_ST):
            for bb in range(BANKS):
                n = int(call_chunks[s, bb]) * P
                blk = idx_arr[call_base[s, bb]:call_base[s, bb] + n]
                wrapped[:, o16:o16 + n // 16] = blk.reshape(-1, 16).T
                o16 += n // 16
        idx_full = np.tile(wrapped, (8, 1))

        maskt = np.zeros((14, tot_idx), np.float16)
        pos = mask_pos[m]
        ld = ldst_s[m]
        for bb in range(7):
            bit = ((ld >> bb) & 1).astype(np.float16)
            maskt[bb, pos] = C_MASK * bit
            maskt[7 + bb, pos] = C_MASK * (1.0 - bit)

        hut = np.zeros((D_FEAT + 14, PAD_NODES), np.float16)
        hut[:D_FEAT, :NPC] = hu[c * NPC:(c + 1) * NPC].T
        hut[D_FEAT:] = np.tile(maskd, (1, N_TILES))

        in_maps.append({
            "hkov": hkov, "idxs": idx_full, "maskt": maskt, "hut": hut,
            "wtaug": wtaug,
        })
    meta = dict(g=g, call_chunks=call_chunks, totc=totc)
    return in_maps, g, meta


_KERNEL_CACHE = {}


def run_gat(hk, hu, W, b, src, dst, repeat=1):
    in_maps, g, meta = prep_inputs_v2(hk, hu, W, b, src, dst)
    key = (tuple(map(tuple, g)), repeat)
    if key not in _KERNEL_CACHE:
        _KERNEL_CACHE[key] = build_v2(g, repeat=repeat)
    nc = _KERNEL_CACHE[key]
    res = run_bass_kernel_spmd(nc, in_maps, core_ids=list(range(N_CORES)))
    out = np.concatenate([res.results[c]["y"] for c in range(N_CORES)],
                         axis=0)
    return np.ascontiguousarray(out, np.float32)


def kernel(hk, hu, W, b, src, dst):
    return run_gat(np.asarray(hk, np.float32), np.asarray(hu, np.float32),
                   np.asarray(W, np.float32), np.asarray(b, np.float32),
                   np.asarray(src), np.asarray(dst))
